# revision 16
# baseline (speedup 1.0000x reference)
"""Trainium2 Bass kernel for nn_MultiHeadAttention_89489938580154.

Multi-head attention with a 64-token memory KV prefix, RoPE on self q/k,
causal self-attention, fp32 I/O.  B=4, L=2048, D=216, H=4, hd=54, M=64.

Sharding: 8 cores = 4 batches x 2 head-groups (2 heads each).  Each core
computes its batch/head-group attention and a partial o_proj; the host sums
the two partials per batch (tensor-parallel all-reduce done at gather time).

v3: bf16 data path + software-pipelined emission so the PE never stalls:
 - all matmul operands bf16 (PSUM accumulation stays fp32); rel err ~6e-3
   vs the 2e-2 gate.  Halves input DMA and enables full-rate matmuls at
   F=128 (V-proj, j3 diagonal chunk, o_proj at F=216).
 - scores(ki) run 3 iterations ahead of AV(ki); AV reads probs from SBUF
   so score PSUM tiles recycle independently of the AV lag.
 - QK-projection of span s+1, V-projection, and o_proj of span s-1 are
   emitted as PE "filler" work inside span s's attention loop (ACT exp is
   slower per-ki than PE scores+AV; fillers absorb the gap).
 - masked-out column ranges of diagonal-chunk scores/exp/AV are trimmed;
   sc/probs tiles are [128, 2(head), 512] so one strided ACT op covers
   both heads and one gpsimd affine_select masks both causal triangles.
 - diagonal AVs are emitted [j1,j2,j3,j0] so the full-width j0 AV is last
   and carries the accumulation-group stop flag; for s>=1 the first AV is
   full-width av(0) (carries start) so av(mem) need not wait on the
   previous span's accumulator drain.
 - PSUM: 2-bank shared ring (qkproj pp/pr, vproj pv, oproj po)
   + 4 banks scores (2 x [128,2,512]) + 2 banks acc (2 x [64,512]) = 8.
"""

import os
import numpy as np

B, L, D = 4, 2048, 216
H, HD, HHD = 4, 54, 27
MEM = 64
NCORES = 8
SPAN = 512
NSPAN = L // SPAN            # 4
KCH = 128                    # kv chunk
NKCH = L // KCH             # 16
ROPE_THETA = 10000.0

_PROGRAM = None


def _build_program(reps=1):
    from concourse import bass, bacc, mybir
    from concourse import tile
    from concourse import library_config

    FP = mybir.dt.float32
    BF = mybir.dt.bfloat16
    Exp = mybir.ActivationFunctionType.Exp
    GE = mybir.AluOpType.is_ge

    nc = bacc.Bacc(None, target_bir_lowering=False, debug=False)

    # ---- DRAM parameters (per-core data, host-prepared, bf16)
    d_xT = nc.dram_tensor("xT", [D, L], BF, kind="ExternalInput").ap()
    d_wq = nc.dram_tensor("wq", [D, 128], BF, kind="ExternalInput").ap()
    d_wk = nc.dram_tensor("wk", [D, 128], BF, kind="ExternalInput").ap()
    d_wqr = nc.dram_tensor("wqr", [D, 128], BF, kind="ExternalInput").ap()
    d_wkr = nc.dram_tensor("wkr", [D, 128], BF, kind="ExternalInput").ap()
    d_wv = nc.dram_tensor("wv", [D, 128], BF, kind="ExternalInput").ap()
    d_wo = nc.dram_tensor("wo", [128, 216], BF, kind="ExternalInput").ap()
    d_cos = nc.dram_tensor("cosT", [128, L], BF, kind="ExternalInput").ap()
    d_sin = nc.dram_tensor("sinT", [128, L], BF, kind="ExternalInput").ap()
    d_mkT = nc.dram_tensor("memkT", [128, 128], BF, kind="ExternalInput").ap()
    d_mv = nc.dram_tensor("memv", [128, MEM], BF, kind="ExternalInput").ap()
    d_out = nc.dram_tensor("outp", [L, D], FP, kind="ExternalOutput").ap()

    with tile.TileContext(nc) as tc:
      nc.gpsimd.load_library(library_config.proxy)
      for _rep in range(reps):
        with tc.tile_pool(name="const", bufs=1) as const, \
             tc.tile_pool(name="work", bufs=2) as work, \
             tc.tile_pool(name="probsp", bufs=6) as probsp, \
             tc.tile_pool(name="mmp", bufs=2, space="PSUM") as mmp, \
             tc.tile_pool(name="scp", bufs=2, space="PSUM") as scp, \
             tc.tile_pool(name="accp", bufs=2, space="PSUM") as accp:

            # ---------- persistent SBUF tiles + input DMA
            # cos/sin arrive host-padded to [128, L]; their DMAs ride the
            # Activation engine's HWDGE queue so they overlap the xT/weight
            # DMAs on the sync queue.
            cos_sb = const.tile([128, L], BF, tag="cos_sb")
            sin_sb = const.tile([128, L], BF, tag="sin_sb")
            for s in range(NSPAN):
                sl = slice(s * SPAN, (s + 1) * SPAN)
                nc.scalar.dma_start(out=cos_sb[:, sl], in_=d_cos[:, sl])
                nc.scalar.dma_start(out=sin_sb[:, sl], in_=d_sin[:, sl])

            wts = {}
            for nm, dr in (("wq", d_wq), ("wqr", d_wqr), ("wk", d_wk),
                           ("wkr", d_wkr)):
                a = const.tile([128, 128], BF, tag=nm + "a", name=nm + "a")
                b = const.tile([88, 128], BF, tag=nm + "b", name=nm + "b")
                nc.sync.dma_start(out=a[:, :], in_=dr[0:128, :])
                nc.sync.dma_start(out=b[:, :], in_=dr[128:216, :])
                wts[nm] = (a, b)

            xTa = const.tile([128, L], BF, tag="xTa")
            xTb = const.tile([88, L], BF, tag="xTb")
            sl0 = slice(0, SPAN)
            nc.sync.dma_start(out=xTa[:, sl0], in_=d_xT[0:128, sl0])
            nc.sync.dma_start(out=xTb[:, sl0], in_=d_xT[128:216, sl0])

            mkT = const.tile([128, 128], BF, tag="mkT")
            nc.sync.dma_start(out=mkT[:, :], in_=d_mkT[:, :])
            mv = const.tile([128, MEM], BF, tag="mv")
            nc.sync.dma_start(out=mv[:, :], in_=d_mv[:, :])

            wva = const.tile([128, 128], BF, tag="wva")
            wvb = const.tile([88, 128], BF, tag="wvb")
            nc.sync.dma_start(out=wva[:, :], in_=d_wv[0:128, :])
            nc.sync.dma_start(out=wvb[:, :], in_=d_wv[128:216, :])

            for s in range(1, NSPAN):
                sl = slice(s * SPAN, (s + 1) * SPAN)
                nc.sync.dma_start(out=xTa[:, sl], in_=d_xT[0:128, sl])
                nc.sync.dma_start(out=xTb[:, sl], in_=d_xT[128:216, sl])

            wo_sb = const.tile([128, 216], BF, tag="wo_sb")
            nc.sync.dma_start(out=wo_sb[:, :], in_=d_wo[:, :])

            QT = const.tile([128, L], BF, tag="QT")
            KT = const.tile([128, L], BF, tag="KT")
            Vg = const.tile([128, NKCH, 128], BF, tag="Vg")
            AN = const.tile([128, L], BF, tag="AN")

            # ---------- emit helpers ------------------------------------
            def emit_qkproj(s, wnm, rnm, dstT, cols=None):
                base = s * SPAN
                c0, c1 = (0, SPAN) if cols is None else cols
                w = c1 - c0
                sl = slice(base + c0, base + c1)
                wa, wb = wts[wnm]
                ra, rb = wts[rnm]
                pp = mmp.tile([128, SPAN], FP, tag="mm", name="pp")
                pr = mmp.tile([128, SPAN], FP, tag="mm", name="pr")
                nc.tensor.matmul(pp[:, 0:w], wa[:, :], xTa[:, sl],
                                 start=True, stop=False)
                nc.tensor.matmul(pp[:, 0:w], wb[:, :], xTb[:, sl],
                                 start=False, stop=True)
                nc.tensor.matmul(pr[:, 0:w], ra[:, :], xTa[:, sl],
                                 start=True, stop=False)
                nc.tensor.matmul(pr[:, 0:w], rb[:, :], xTb[:, sl],
                                 start=False, stop=True)
                t1 = work.tile([128, SPAN], FP, tag="ropet1", name="t1")
                t2 = work.tile([128, SPAN], FP, tag="ropet2", name="t2")
                nc.vector.tensor_mul(t1[:, 0:w], pr[:, 0:w], sin_sb[:, sl])
                nc.vector.tensor_mul(t2[:, 0:w], pp[:, 0:w], cos_sb[:, sl])
                nc.vector.tensor_add(dstT[:, sl], t1[:, 0:w], t2[:, 0:w])

            def emit_vproj(k):
                sl = slice(k * KCH, (k + 1) * KCH)
                pv = mmp.tile([128, SPAN], FP, tag="mm", name="pv")
                nc.tensor.matmul(pv[:, 0:128], xTa[:, sl], wva[:, :],
                                 start=True, stop=False)
                nc.tensor.matmul(pv[:, 0:128], xTb[:, sl], wvb[:, :],
                                 start=False, stop=True)
                nc.vector.tensor_copy(Vg[:, k, :], pv[:, 0:128])
                nc.gpsimd.memset(Vg[:, k, 32::64].bitcast(BF), 1.0)

            def emit_oproj(qt):
                po = mmp.tile([128, SPAN], FP, tag="mm", name="po")
                nc.tensor.matmul(po[:, 0:216], AN[:, qt:qt + 128],
                                 wo_sb[:, :], start=True, stop=True)
                ost = work.tile([128, D], FP, tag="ost", name="ost")
                nc.vector.tensor_copy(ost[:, :], po[:, 0:D])
                nc.sync.dma_start(out=d_out[qt:qt + 128, :], in_=ost[:, :])

            def trim(s, ki):
                """(off, F) of the computed q-column range per head."""
                j = ki - 4 * s
                if ki < 0 or j < 0 or j == 0:
                    return 0, SPAN
                return 128 * j, SPAN - 128 * j     # j in (1, 2, 3)

            def emit_scores(s, ki, sc):
                qbase = s * SPAN
                off, F = trim(s, ki)
                if ki < 0 and s == 0:
                    # lead-in: half-width so the ACT engine starts sooner
                    for c in (0, 256):
                        qsl = slice(qbase + c, qbase + c + 256)
                        nc.tensor.matmul(sc[:, 0, c:c + 256], mkT[0:118, :],
                                         QT[0:118, qsl],
                                         start=True, stop=True)
                    return
                if ki < 0:
                    # mkT is block-diagonal (h0 keys in cols 0:64 live only on
                    # h0 dim-rows, h1 keys in cols 64:128 on h1 dim-rows), so
                    # one matmul yields both heads' memory scores stacked on
                    # the kv partition axis: rows 0:64 = h0, 64:128 = h1.
                    qsl = slice(qbase, qbase + SPAN)
                    nc.tensor.matmul(sc[:, 0, :], mkT[0:118, :],
                                     QT[0:118, qsl], start=True, stop=True)
                    return
                for h in range(2):
                    hq = slice(64 * h, 64 * h + HD)
                    qsl = slice(qbase + off, qbase + off + F)
                    ksl = slice(ki * KCH, (ki + 1) * KCH)
                    nc.tensor.matmul(sc[:, h, off:off + F], KT[hq, ksl],
                                     QT[hq, qsl], start=True, stop=True)

            def emit_exp_mask(s, ki, sc, pb):
                if ki < 0:
                    if s == 0:
                        for c in (0, 256):
                            nc.scalar.activation(pb[:, 0, c:c + 256],
                                                 sc[:, 0, c:c + 256], Exp)
                    else:
                        nc.scalar.activation(pb[:, 0, :], sc[:, 0, :], Exp)
                    return
                off, F = trim(s, ki)
                nc.scalar.activation(pb[:, :, off:off + F],
                                     sc[:, :, off:off + F], Exp)
                if ki >= 4 * s:
                    j = ki - 4 * s
                    # zero the upper (future) triangle of the diagonal
                    # 128x128 block, both heads in one op
                    nc.gpsimd.affine_select(
                        out=pb[:, :, 128 * j:128 * j + 128],
                        in_=pb[:, :, 128 * j:128 * j + 128],
                        compare_op=GE, fill=0.0, base=0,
                        pattern=[[0, 2], [1, 128]], channel_multiplier=-1)

            def emit_av(s, ki, pb, accs, first, last):
                off, F = trim(s, ki)
                for h in range(2):
                    if ki < 0:
                        # packed mem probs: h0 on kv partitions 0:64,
                        # h1 on 64:128, all within pb[:, 0, :]
                        nc.tensor.matmul(accs[h][0:64, 0:SPAN],
                                         mv[64 * h:64 * h + 64, :],
                                         pb[64 * h:64 * h + 64, 0, 0:SPAN],
                                         start=first, stop=False,
                                         skip_group_check=True)
                    else:
                        nc.tensor.matmul(accs[h][0:64, off:off + F],
                                         Vg[:, ki, 64 * h:64 * h + 64],
                                         pb[:, h, off:off + F],
                                         start=first, stop=last,
                                         skip_group_check=True)

            def emit_norm(s, accs, half):
                csl = slice(half * 256, half * 256 + 256)
                qsl = slice(s * SPAN + half * 256, s * SPAN + half * 256 + 256)
                for h in range(2):
                    den = work.tile([1, 256], FP, tag="den", name="den")
                    nc.vector.reciprocal(den[:, :], accs[h][32:33, csl])
                    denb = work.tile([64, 256], FP, tag="denb", name="denb")
                    nc.gpsimd.partition_broadcast(denb[:, :], den[:, :])
                    nc.vector.tensor_mul(AN[64 * h:64 * h + 64, qsl],
                                         accs[h][0:64, csl], denb[:, :])

            # ---------- span-pipelined main loop ------------------------
            emit_qkproj(0, "wq", "wqr", QT, cols=(0, 256))
            emit_qkproj(0, "wq", "wqr", QT, cols=(256, 512))
            emit_qkproj(0, "wk", "wkr", KT)
            emit_vproj(0)

            AVLAG = 3
            for s in range(NSPAN):
                fillers = []
                for k in range(4 * s + (1 if s == 0 else 0), 4 * s + 4):
                    fillers.append(lambda k=k: emit_vproj(k))
                if s < NSPAN - 1:
                    fillers.append(lambda s=s: emit_qkproj(s + 1, "wq", "wqr", QT))
                    fillers.append(lambda s=s: emit_qkproj(s + 1, "wk", "wkr", KT))
                if s >= 1:
                    for t in range(4):
                        qt = (s - 1) * SPAN + t * 128
                        fillers.append(lambda qt=qt: emit_oproj(qt))

                kis = [-1] + list(range(4 * s + 4))
                # AV order: full-width av(0) first (carries start) when it
                # exists, then natural order.  The accumulation-group stop
                # rides the last (j3) AV, which only touches columns
                # [384,512) -- so the half-span norms depend only on the
                # earlier AVs and overlap the diagonal tail.
                if s == 0:
                    av_order = [-1, 0, 1, 2, 3]
                else:
                    av_order = [0, -1] + list(range(1, 4 * s + 4))
                n = len(kis)
                accs = [accp.tile([64, SPAN], FP, tag="acc", name="acc")
                        for _ in range(2)]
                pbs = {}
                for idx, ki in enumerate(kis):
                    sc = scp.tile([128, 2, SPAN], FP, tag="sc", name="sc")
                    emit_scores(s, ki, sc)
                    pb = probsp.tile([128, 2, SPAN], BF, tag="probs",
                                     name="pb")
                    pbs[ki] = pb
                    emit_exp_mask(s, ki, sc, pb)
                    if fillers:
                        fillers.pop(0)()
                    if idx >= AVLAG:
                        aki = av_order[idx - AVLAG]
                        emit_av(s, aki, pbs[aki], accs,
                                first=(aki == av_order[0]),
                                last=(aki == 4 * s + 3))
                for i in range(n - AVLAG, n):
                    aki = av_order[i]
                    emit_av(s, aki, pbs[aki], accs,
                            first=(aki == av_order[0]),
                            last=(aki == 4 * s + 3))
                for f in fillers:
                    f()
                if s < NSPAN - 1:
                    emit_norm(s, accs, 0)
                    emit_norm(s, accs, 1)
                else:
                    # tail: half-span norms feed o_proj tiles immediately
                    for half in (0, 1):
                        emit_norm(s, accs, half)
                        for t in (2 * half, 2 * half + 1):
                            emit_oproj(s * SPAN + t * 128)

    nc.compile()
    return nc


def _host_inputs(x, mem_k, mem_v, Wqkv, Wo):
    """Build the per-core input maps (host-side sharding + layout prep)."""
    import ml_dtypes
    f32 = np.float32
    bf16 = ml_dtypes.bfloat16
    x = np.asarray(x, f32)
    mem_k = np.asarray(mem_k, f32)
    mem_v = np.asarray(mem_v, f32)
    Wqkv = np.asarray(Wqkv, f32)
    Wo = np.asarray(Wo, f32)

    Wq, Wk, Wv = Wqkv[:, 0:D], Wqkv[:, D:2 * D], Wqkv[:, 2 * D:3 * D]
    scale = f32(HD ** -0.5)

    # RoPE tables, host-padded to [128, 2048]: rows 0:54 and 64:118 hold the
    # per-head tables (identical), pad rows zeroed; sign of rotate_half
    # folded into sinT
    inv = 1.0 / (ROPE_THETA ** (np.arange(0, HD, 2, dtype=np.float64) / HD))
    t = np.arange(L, dtype=np.float64)
    fr = np.outer(t, inv)                       # [L, 27]
    emb = np.concatenate([fr, fr], axis=-1)     # [L, 54]
    cos54 = np.ascontiguousarray(np.cos(emb).T).astype(f32)
    sin54 = np.ascontiguousarray(np.sin(emb).T).astype(f32)
    sin54[:HHD] *= -1.0
    cosT = np.zeros((128, L), f32)
    sinT = np.zeros((128, L), f32)
    for base in (0, 64):
        cosT[base:base + HD] = cos54
        sinT[base:base + HD] = sin54
    cosT = cosT.astype(bf16)
    sinT = sinT.astype(bf16)

    rotperm = np.concatenate([np.arange(HHD, HD), np.arange(0, HHD)])

    in_maps = []
    for c in range(NCORES):
        b, hg = c // 2, c % 2
        c0 = hg * 2 * HD                        # first head-dim col

        def padw(w, sc=None):
            out = np.zeros((D, 128), f32)
            blk = w[:, c0:c0 + 2 * HD]
            if sc is not None:
                blk = blk * sc
            out[:, 0:HD] = blk[:, 0:HD]
            out[:, 64:64 + HD] = blk[:, HD:2 * HD]
            return out

        wq_p = padw(Wq, scale)
        wk_p = padw(Wk)
        wqr_p = np.zeros_like(wq_p)
        wkr_p = np.zeros_like(wk_p)
        for base in (0, 64):
            wqr_p[:, base:base + HD] = wq_p[:, base:base + HD][:, rotperm]
            wkr_p[:, base:base + HD] = wk_p[:, base:base + HD][:, rotperm]

        # per-head 64-col block: [V d0:32 | ones-slot | V d32:54 | zeros]
        wv_p = np.zeros((D, 128), f32)
        for hh in range(2):
            hcol = c0 + hh * HD
            wv_p[:, 64 * hh + 0:64 * hh + 32] = Wv[:, hcol:hcol + 32]
            wv_p[:, 64 * hh + 33:64 * hh + 55] = Wv[:, hcol + 32:hcol + HD]

        # rows match AN layout: [d0:32 | dead | d32:54 | dead] per head
        wo_p = np.zeros((128, 216), f32)
        for hh in range(2):
            hrow = c0 + hh * HD
            wo_p[64 * hh + 0:64 * hh + 32, :] = Wo[hrow:hrow + 32, :]
            wo_p[64 * hh + 33:64 * hh + 55, :] = Wo[hrow + 32:hrow + HD, :]

        # block-diagonal: h0 keys in cols 0:64 on h0 dim-rows only, h1 keys
        # in cols 64:128 on h1 dim-rows only -> one matmul computes both
        # heads' memory scores stacked on the kv partition axis
        mkT_p = np.zeros((128, 128), f32)
        mkT_p[0:HD, 0:MEM] = mem_k[b][:, c0:c0 + HD].T
        mkT_p[64:64 + HD, MEM:2 * MEM] = mem_k[b][:, c0 + HD:c0 + 2 * HD].T

        # [128, 64]: head hh's mem-V block [kv, vd] on kv-partitions
        # 64*hh:64*hh+64, matching the packed mem-probs partition ranges;
        # columns follow the per-head [32 v | ones | 22 v | 9 dead] layout
        mv_p = np.zeros((128, MEM), f32)
        for hh in range(2):
            hcol = c0 + hh * HD
            base = 64 * hh
            mv_p[base:base + MEM, 0:32] = mem_v[b][:, hcol:hcol + 32]
            mv_p[base:base + MEM, 32] = 1.0
            mv_p[base:base + MEM, 33:55] = mem_v[b][:, hcol + 32:hcol + HD]

        in_maps.append({
            "xT": np.ascontiguousarray(x[b].T).astype(bf16),
            "wq": wq_p.astype(bf16), "wk": wk_p.astype(bf16),
            "wqr": wqr_p.astype(bf16), "wkr": wkr_p.astype(bf16),
            "wv": wv_p.astype(bf16), "wo": wo_p.astype(bf16),
            "cosT": cosT, "sinT": sinT,
            "memkT": mkT_p.astype(bf16), "memv": mv_p.astype(bf16),
        })
    return in_maps


def get_program():
    global _PROGRAM
    if _PROGRAM is None:
        _PROGRAM = _build_program()
    return _PROGRAM


def kernel(x, mem_k, mem_v, attention_mask, Wqkv, Wo):
    from concourse.bass_utils import run_bass_kernel_spmd

    nc = get_program()
    in_maps = _host_inputs(x, mem_k, mem_v, Wqkv, Wo)
    trace = bool(int(os.environ.get("KB_TRACE", "0")))
    res = run_bass_kernel_spmd(nc, in_maps, core_ids=list(range(NCORES)),
                               trace=trace)
    if trace and res.exec_time_ns is not None:
        print(f"HW exec time: {res.exec_time_ns} ns")
    parts = [res.results[c]["outp"] for c in range(NCORES)]
    out = np.stack([parts[2 * b] + parts[2 * b + 1] for b in range(B)])
    return out.astype(np.float32)


# revision 17
# speedup vs baseline: 2.0600x; 2.0600x over previous
"""Trainium2 Bass kernel for nn_MultiHeadAttention_89489938580154.

Multi-head attention with a 64-token memory KV prefix, RoPE on self q/k,
causal self-attention, fp32 I/O.  B=4, L=2048, D=216, H=4, hd=54, M=64.

Sharding: 8 cores = 4 batches x 2 head-groups (2 heads each).  Each core
computes its batch/head-group attention and a partial o_proj; the host sums
the two partials per batch (tensor-parallel all-reduce done at gather time).

bf16 data path + software-pipelined emission so the PE never stalls:
 - all matmul operands bf16 (PSUM accumulation stays fp32); rel err ~6e-3
   vs the 2e-2 gate.  Halves input DMA and enables full-rate matmuls at
   F=128 (V-proj, j3 diagonal chunk, o_proj at F=216).
 - scores(ki) run 3 iterations ahead of AV(ki); AV reads probs from SBUF
   so score PSUM tiles recycle independently of the AV lag.
 - QK-projection of span s+1, V-projection, and o_proj of span s-1 are
   emitted as PE "filler" work inside span s's attention loop (ACT exp is
   slower per-ki than PE scores+AV; fillers absorb the gap).
 - masked-out column ranges of diagonal-chunk scores/exp/AV are trimmed;
   sc/probs tiles are [128, 2(head), 512] so one strided ACT op covers
   both heads and one gpsimd affine_select masks both causal triangles.
 - both heads' memory-chunk scores come from ONE matmul (block-diagonal
   mkT, heads stacked on the kv partition axis), halving that chunk's
   PE and ACT cost.
 - AVs run in natural ki order with the accumulation-group stop on the
   last (j3) AV, which only touches columns [384,512): the half-span
   normalizations depend only on earlier AVs and overlap the diagonal
   tail.  For s>=1 the first AV is full-width av(0) (carries start) so
   av(mem) need not wait on the previous span's accumulator drain.
 - span 0's q-projection / first scores / first exp run at half width so
   the ACT engine starts ~2us sooner; the last span's norm->o_proj chain
   is emitted per half-span to shrink the serial tail.
 - PSUM: 2-bank shared ring (qkproj pp/pr, vproj pv, oproj po)
   + 4 banks scores (2 x [128,2,512]) + 2 banks acc (2 x [64,512]) = 8.
"""

import os
import numpy as np

B, L, D = 4, 2048, 216
H, HD, HHD = 4, 54, 27
MEM = 64
NCORES = 8
SPAN = 512
NSPAN = L // SPAN            # 4
KCH = 128                    # kv chunk
NKCH = L // KCH             # 16
ROPE_THETA = 10000.0

_PROGRAM = None


def _build_program(reps=1):
    from concourse import bass, bacc, mybir
    from concourse import tile
    from concourse import library_config

    FP = mybir.dt.float32
    BF = mybir.dt.bfloat16
    Exp = mybir.ActivationFunctionType.Exp
    GE = mybir.AluOpType.is_ge

    nc = bacc.Bacc(None, target_bir_lowering=False, debug=False)

    # ---- DRAM parameters (per-core data, host-prepared, bf16)
    d_xT = nc.dram_tensor("xT", [D, L], BF, kind="ExternalInput").ap()
    d_wq = nc.dram_tensor("wq", [D, 128], BF, kind="ExternalInput").ap()
    d_wk = nc.dram_tensor("wk", [D, 128], BF, kind="ExternalInput").ap()
    d_wqr = nc.dram_tensor("wqr", [D, 128], BF, kind="ExternalInput").ap()
    d_wkr = nc.dram_tensor("wkr", [D, 128], BF, kind="ExternalInput").ap()
    d_wv = nc.dram_tensor("wv", [D, 128], BF, kind="ExternalInput").ap()
    d_wo = nc.dram_tensor("wo", [128, 216], BF, kind="ExternalInput").ap()
    d_cos = nc.dram_tensor("cosT", [128, L], BF, kind="ExternalInput").ap()
    d_sin = nc.dram_tensor("sinT", [128, L], BF, kind="ExternalInput").ap()
    d_mkT = nc.dram_tensor("memkT", [128, 128], BF, kind="ExternalInput").ap()
    d_mv = nc.dram_tensor("memv", [128, MEM], BF, kind="ExternalInput").ap()
    d_out = nc.dram_tensor("outp", [L, D], FP, kind="ExternalOutput").ap()

    with tile.TileContext(nc) as tc:
      nc.gpsimd.load_library(library_config.proxy)
      for _rep in range(reps):
        with tc.tile_pool(name="const", bufs=1) as const, \
             tc.tile_pool(name="work", bufs=2) as work, \
             tc.tile_pool(name="probsp", bufs=6) as probsp, \
             tc.tile_pool(name="mmp", bufs=2, space="PSUM") as mmp, \
             tc.tile_pool(name="scp", bufs=2, space="PSUM") as scp, \
             tc.tile_pool(name="accp", bufs=2, space="PSUM") as accp:

            # ---------- persistent SBUF tiles + input DMA
            # cos/sin arrive host-padded to [128, L]; their DMAs ride the
            # Activation engine's HWDGE queue so they overlap the xT/weight
            # DMAs on the sync queue.
            cos_sb = const.tile([128, L], BF, tag="cos_sb")
            sin_sb = const.tile([128, L], BF, tag="sin_sb")
            for s in range(NSPAN):
                sl = slice(s * SPAN, (s + 1) * SPAN)
                nc.scalar.dma_start(out=cos_sb[:, sl], in_=d_cos[:, sl])
                nc.scalar.dma_start(out=sin_sb[:, sl], in_=d_sin[:, sl])

            wts = {}
            for nm, dr in (("wq", d_wq), ("wqr", d_wqr), ("wk", d_wk),
                           ("wkr", d_wkr)):
                a = const.tile([128, 128], BF, tag=nm + "a", name=nm + "a")
                b = const.tile([88, 128], BF, tag=nm + "b", name=nm + "b")
                nc.sync.dma_start(out=a[:, :], in_=dr[0:128, :])
                nc.sync.dma_start(out=b[:, :], in_=dr[128:216, :])
                wts[nm] = (a, b)

            xTa = const.tile([128, L], BF, tag="xTa")
            xTb = const.tile([88, L], BF, tag="xTb")
            sl0 = slice(0, SPAN)
            nc.sync.dma_start(out=xTa[:, sl0], in_=d_xT[0:128, sl0])
            nc.sync.dma_start(out=xTb[:, sl0], in_=d_xT[128:216, sl0])

            mkT = const.tile([128, 128], BF, tag="mkT")
            nc.sync.dma_start(out=mkT[:, :], in_=d_mkT[:, :])
            mv = const.tile([128, MEM], BF, tag="mv")
            nc.sync.dma_start(out=mv[:, :], in_=d_mv[:, :])

            wva = const.tile([128, 128], BF, tag="wva")
            wvb = const.tile([88, 128], BF, tag="wvb")
            nc.sync.dma_start(out=wva[:, :], in_=d_wv[0:128, :])
            nc.sync.dma_start(out=wvb[:, :], in_=d_wv[128:216, :])

            for s in range(1, NSPAN):
                sl = slice(s * SPAN, (s + 1) * SPAN)
                nc.sync.dma_start(out=xTa[:, sl], in_=d_xT[0:128, sl])
                nc.sync.dma_start(out=xTb[:, sl], in_=d_xT[128:216, sl])

            wo_sb = const.tile([128, 216], BF, tag="wo_sb")
            nc.sync.dma_start(out=wo_sb[:, :], in_=d_wo[:, :])

            QT = const.tile([128, L], BF, tag="QT")
            KT = const.tile([128, L], BF, tag="KT")
            Vg = const.tile([128, NKCH, 128], BF, tag="Vg")
            AN = const.tile([128, L], BF, tag="AN")

            # ---------- emit helpers ------------------------------------
            def emit_qkproj(s, wnm, rnm, dstT, cols=None):
                base = s * SPAN
                c0, c1 = (0, SPAN) if cols is None else cols
                w = c1 - c0
                sl = slice(base + c0, base + c1)
                wa, wb = wts[wnm]
                ra, rb = wts[rnm]
                pp = mmp.tile([128, SPAN], FP, tag="mm", name="pp")
                pr = mmp.tile([128, SPAN], FP, tag="mm", name="pr")
                nc.tensor.matmul(pp[:, 0:w], wa[:, :], xTa[:, sl],
                                 start=True, stop=False)
                nc.tensor.matmul(pp[:, 0:w], wb[:, :], xTb[:, sl],
                                 start=False, stop=True)
                nc.tensor.matmul(pr[:, 0:w], ra[:, :], xTa[:, sl],
                                 start=True, stop=False)
                nc.tensor.matmul(pr[:, 0:w], rb[:, :], xTb[:, sl],
                                 start=False, stop=True)
                t1 = work.tile([128, SPAN], FP, tag="ropet1", name="t1")
                t2 = work.tile([128, SPAN], FP, tag="ropet2", name="t2")
                nc.vector.tensor_mul(t1[:, 0:w], pr[:, 0:w], sin_sb[:, sl])
                nc.vector.tensor_mul(t2[:, 0:w], pp[:, 0:w], cos_sb[:, sl])
                nc.vector.tensor_add(dstT[:, sl], t1[:, 0:w], t2[:, 0:w])

            def emit_vproj(k):
                sl = slice(k * KCH, (k + 1) * KCH)
                pv = mmp.tile([128, SPAN], FP, tag="mm", name="pv")
                nc.tensor.matmul(pv[:, 0:128], xTa[:, sl], wva[:, :],
                                 start=True, stop=False)
                nc.tensor.matmul(pv[:, 0:128], xTb[:, sl], wvb[:, :],
                                 start=False, stop=True)
                nc.vector.tensor_copy(Vg[:, k, :], pv[:, 0:128])
                nc.gpsimd.memset(Vg[:, k, 32::64].bitcast(BF), 1.0)

            def emit_oproj(qt):
                po = mmp.tile([128, SPAN], FP, tag="mm", name="po")
                nc.tensor.matmul(po[:, 0:216], AN[:, qt:qt + 128],
                                 wo_sb[:, :], start=True, stop=True)
                ost = work.tile([128, D], FP, tag="ost", name="ost")
                nc.vector.tensor_copy(ost[:, :], po[:, 0:D])
                nc.sync.dma_start(out=d_out[qt:qt + 128, :], in_=ost[:, :])

            def trim(s, ki):
                """(off, F) of the computed q-column range per head."""
                j = ki - 4 * s
                if ki < 0 or j < 0 or j == 0:
                    return 0, SPAN
                return 128 * j, SPAN - 128 * j     # j in (1, 2, 3)

            def emit_scores(s, ki, sc):
                qbase = s * SPAN
                off, F = trim(s, ki)
                if ki < 0 and s == 0:
                    # lead-in: half-width so the ACT engine starts sooner
                    for c in (0, 256):
                        qsl = slice(qbase + c, qbase + c + 256)
                        nc.tensor.matmul(sc[:, 0, c:c + 256], mkT[0:118, :],
                                         QT[0:118, qsl],
                                         start=True, stop=True)
                    return
                if ki < 0:
                    # mkT is block-diagonal (h0 keys in cols 0:64 live only on
                    # h0 dim-rows, h1 keys in cols 64:128 on h1 dim-rows), so
                    # one matmul yields both heads' memory scores stacked on
                    # the kv partition axis: rows 0:64 = h0, 64:128 = h1.
                    qsl = slice(qbase, qbase + SPAN)
                    nc.tensor.matmul(sc[:, 0, :], mkT[0:118, :],
                                     QT[0:118, qsl], start=True, stop=True)
                    return
                for h in range(2):
                    hq = slice(64 * h, 64 * h + HD)
                    qsl = slice(qbase + off, qbase + off + F)
                    ksl = slice(ki * KCH, (ki + 1) * KCH)
                    nc.tensor.matmul(sc[:, h, off:off + F], KT[hq, ksl],
                                     QT[hq, qsl], start=True, stop=True)

            def emit_exp_mask(s, ki, sc, pb):
                if ki < 0:
                    if s == 0:
                        for c in (0, 256):
                            nc.scalar.activation(pb[:, 0, c:c + 256],
                                                 sc[:, 0, c:c + 256], Exp)
                    else:
                        nc.scalar.activation(pb[:, 0, :], sc[:, 0, :], Exp)
                    return
                off, F = trim(s, ki)
                nc.scalar.activation(pb[:, :, off:off + F],
                                     sc[:, :, off:off + F], Exp)
                if ki >= 4 * s:
                    j = ki - 4 * s
                    # zero the upper (future) triangle of the diagonal
                    # 128x128 block, both heads in one op
                    nc.gpsimd.affine_select(
                        out=pb[:, :, 128 * j:128 * j + 128],
                        in_=pb[:, :, 128 * j:128 * j + 128],
                        compare_op=GE, fill=0.0, base=0,
                        pattern=[[0, 2], [1, 128]], channel_multiplier=-1)

            def emit_av(s, ki, pb, accs, first, last):
                off, F = trim(s, ki)
                for h in range(2):
                    if ki < 0:
                        # packed mem probs: h0 on kv partitions 0:64,
                        # h1 on 64:128, all within pb[:, 0, :]
                        nc.tensor.matmul(accs[h][0:64, 0:SPAN],
                                         mv[64 * h:64 * h + 64, :],
                                         pb[64 * h:64 * h + 64, 0, 0:SPAN],
                                         start=first, stop=False,
                                         skip_group_check=True)
                    else:
                        nc.tensor.matmul(accs[h][0:64, off:off + F],
                                         Vg[:, ki, 64 * h:64 * h + 64],
                                         pb[:, h, off:off + F],
                                         start=first, stop=last,
                                         skip_group_check=True)

            def emit_norm(s, accs, half):
                csl = slice(half * 256, half * 256 + 256)
                qsl = slice(s * SPAN + half * 256, s * SPAN + half * 256 + 256)
                for h in range(2):
                    den = work.tile([1, 256], FP, tag="den", name="den")
                    nc.vector.reciprocal(den[:, :], accs[h][32:33, csl])
                    denb = work.tile([64, 256], FP, tag="denb", name="denb")
                    nc.gpsimd.partition_broadcast(denb[:, :], den[:, :])
                    nc.vector.tensor_mul(AN[64 * h:64 * h + 64, qsl],
                                         accs[h][0:64, csl], denb[:, :])

            # ---------- span-pipelined main loop ------------------------
            emit_qkproj(0, "wq", "wqr", QT, cols=(0, 256))
            emit_qkproj(0, "wq", "wqr", QT, cols=(256, 512))
            emit_qkproj(0, "wk", "wkr", KT)
            emit_vproj(0)

            AVLAG = 3
            for s in range(NSPAN):
                fillers = []
                for k in range(4 * s + (1 if s == 0 else 0), 4 * s + 4):
                    fillers.append(lambda k=k: emit_vproj(k))
                if s < NSPAN - 1:
                    fillers.append(lambda s=s: emit_qkproj(s + 1, "wq", "wqr", QT))
                    fillers.append(lambda s=s: emit_qkproj(s + 1, "wk", "wkr", KT))
                if s >= 1:
                    for t in range(4):
                        qt = (s - 1) * SPAN + t * 128
                        fillers.append(lambda qt=qt: emit_oproj(qt))

                kis = [-1] + list(range(4 * s + 4))
                # AV order: full-width av(0) first (carries start) when it
                # exists, then natural order.  The accumulation-group stop
                # rides the last (j3) AV, which only touches columns
                # [384,512) -- so the half-span norms depend only on the
                # earlier AVs and overlap the diagonal tail.
                if s == 0:
                    av_order = [-1, 0, 1, 2, 3]
                else:
                    av_order = [0, -1] + list(range(1, 4 * s + 4))
                n = len(kis)
                accs = [accp.tile([64, SPAN], FP, tag="acc", name="acc")
                        for _ in range(2)]
                pbs = {}
                for idx, ki in enumerate(kis):
                    sc = scp.tile([128, 2, SPAN], FP, tag="sc", name="sc")
                    emit_scores(s, ki, sc)
                    pb = probsp.tile([128, 2, SPAN], BF, tag="probs",
                                     name="pb")
                    pbs[ki] = pb
                    emit_exp_mask(s, ki, sc, pb)
                    if fillers:
                        fillers.pop(0)()
                    if idx >= AVLAG:
                        aki = av_order[idx - AVLAG]
                        emit_av(s, aki, pbs[aki], accs,
                                first=(aki == av_order[0]),
                                last=(aki == 4 * s + 3))
                for i in range(n - AVLAG, n):
                    aki = av_order[i]
                    emit_av(s, aki, pbs[aki], accs,
                            first=(aki == av_order[0]),
                            last=(aki == 4 * s + 3))
                for f in fillers:
                    f()
                if s < NSPAN - 1:
                    emit_norm(s, accs, 0)
                    emit_norm(s, accs, 1)
                else:
                    # tail: half-span norms feed o_proj tiles immediately
                    for half in (0, 1):
                        emit_norm(s, accs, half)
                        for t in (2 * half, 2 * half + 1):
                            emit_oproj(s * SPAN + t * 128)

    nc.compile()
    return nc


def _host_inputs(x, mem_k, mem_v, Wqkv, Wo):
    """Build the per-core input maps (host-side sharding + layout prep)."""
    import ml_dtypes
    f32 = np.float32
    bf16 = ml_dtypes.bfloat16
    x = np.asarray(x, f32)
    mem_k = np.asarray(mem_k, f32)
    mem_v = np.asarray(mem_v, f32)
    Wqkv = np.asarray(Wqkv, f32)
    Wo = np.asarray(Wo, f32)

    Wq, Wk, Wv = Wqkv[:, 0:D], Wqkv[:, D:2 * D], Wqkv[:, 2 * D:3 * D]
    scale = f32(HD ** -0.5)

    # RoPE tables, host-padded to [128, 2048]: rows 0:54 and 64:118 hold the
    # per-head tables (identical), pad rows zeroed; sign of rotate_half
    # folded into sinT
    inv = 1.0 / (ROPE_THETA ** (np.arange(0, HD, 2, dtype=np.float64) / HD))
    t = np.arange(L, dtype=np.float64)
    fr = np.outer(t, inv)                       # [L, 27]
    emb = np.concatenate([fr, fr], axis=-1)     # [L, 54]
    cos54 = np.ascontiguousarray(np.cos(emb).T).astype(f32)
    sin54 = np.ascontiguousarray(np.sin(emb).T).astype(f32)
    sin54[:HHD] *= -1.0
    cosT = np.zeros((128, L), f32)
    sinT = np.zeros((128, L), f32)
    for base in (0, 64):
        cosT[base:base + HD] = cos54
        sinT[base:base + HD] = sin54
    cosT = cosT.astype(bf16)
    sinT = sinT.astype(bf16)

    rotperm = np.concatenate([np.arange(HHD, HD), np.arange(0, HHD)])

    in_maps = []
    for c in range(NCORES):
        b, hg = c // 2, c % 2
        c0 = hg * 2 * HD                        # first head-dim col

        def padw(w, sc=None):
            out = np.zeros((D, 128), f32)
            blk = w[:, c0:c0 + 2 * HD]
            if sc is not None:
                blk = blk * sc
            out[:, 0:HD] = blk[:, 0:HD]
            out[:, 64:64 + HD] = blk[:, HD:2 * HD]
            return out

        wq_p = padw(Wq, scale)
        wk_p = padw(Wk)
        wqr_p = np.zeros_like(wq_p)
        wkr_p = np.zeros_like(wk_p)
        for base in (0, 64):
            wqr_p[:, base:base + HD] = wq_p[:, base:base + HD][:, rotperm]
            wkr_p[:, base:base + HD] = wk_p[:, base:base + HD][:, rotperm]

        # per-head 64-col block: [V d0:32 | ones-slot | V d32:54 | zeros]
        wv_p = np.zeros((D, 128), f32)
        for hh in range(2):
            hcol = c0 + hh * HD
            wv_p[:, 64 * hh + 0:64 * hh + 32] = Wv[:, hcol:hcol + 32]
            wv_p[:, 64 * hh + 33:64 * hh + 55] = Wv[:, hcol + 32:hcol + HD]

        # rows match AN layout: [d0:32 | dead | d32:54 | dead] per head
        wo_p = np.zeros((128, 216), f32)
        for hh in range(2):
            hrow = c0 + hh * HD
            wo_p[64 * hh + 0:64 * hh + 32, :] = Wo[hrow:hrow + 32, :]
            wo_p[64 * hh + 33:64 * hh + 55, :] = Wo[hrow + 32:hrow + HD, :]

        # block-diagonal: h0 keys in cols 0:64 on h0 dim-rows only, h1 keys
        # in cols 64:128 on h1 dim-rows only -> one matmul computes both
        # heads' memory scores stacked on the kv partition axis
        mkT_p = np.zeros((128, 128), f32)
        mkT_p[0:HD, 0:MEM] = mem_k[b][:, c0:c0 + HD].T
        mkT_p[64:64 + HD, MEM:2 * MEM] = mem_k[b][:, c0 + HD:c0 + 2 * HD].T

        # [128, 64]: head hh's mem-V block [kv, vd] on kv-partitions
        # 64*hh:64*hh+64, matching the packed mem-probs partition ranges;
        # columns follow the per-head [32 v | ones | 22 v | 9 dead] layout
        mv_p = np.zeros((128, MEM), f32)
        for hh in range(2):
            hcol = c0 + hh * HD
            base = 64 * hh
            mv_p[base:base + MEM, 0:32] = mem_v[b][:, hcol:hcol + 32]
            mv_p[base:base + MEM, 32] = 1.0
            mv_p[base:base + MEM, 33:55] = mem_v[b][:, hcol + 32:hcol + HD]

        in_maps.append({
            "xT": np.ascontiguousarray(x[b].T).astype(bf16),
            "wq": wq_p.astype(bf16), "wk": wk_p.astype(bf16),
            "wqr": wqr_p.astype(bf16), "wkr": wkr_p.astype(bf16),
            "wv": wv_p.astype(bf16), "wo": wo_p.astype(bf16),
            "cosT": cosT, "sinT": sinT,
            "memkT": mkT_p.astype(bf16), "memv": mv_p.astype(bf16),
        })
    return in_maps


def get_program():
    global _PROGRAM
    if _PROGRAM is None:
        _PROGRAM = _build_program()
    return _PROGRAM


def kernel(x, mem_k, mem_v, attention_mask, Wqkv, Wo):
    from concourse.bass_utils import run_bass_kernel_spmd

    nc = get_program()
    in_maps = _host_inputs(x, mem_k, mem_v, Wqkv, Wo)
    trace = bool(int(os.environ.get("KB_TRACE", "0")))
    res = run_bass_kernel_spmd(nc, in_maps, core_ids=list(range(NCORES)),
                               trace=trace)
    if trace and res.exec_time_ns is not None:
        print(f"HW exec time: {res.exec_time_ns} ns")
    parts = [res.results[c]["outp"] for c in range(NCORES)]
    out = np.stack([parts[2 * b] + parts[2 * b + 1] for b in range(B)])
    return out.astype(np.float32)
